# revision 2
# baseline (speedup 1.0000x reference)
"""Trainium2 Bass kernel for nn_AttentionModule (channel self-attention).

Reference computation (per batch sample b, with x: [C=512, N=4096]):
    q   = w1 @ x + b1                     # [64, 4096]
    att = softmax(q @ q.T, axis=-1)       # [64, 64]
    out = att @ q                         # [64, 4096]
    y   = w2 @ out + b2 + x               # [512, 4096]

Sharding: data-parallel over batch. B=16 samples, 8 cores, 2 samples/core.
Small weights (w1,b1,w2,b2) replicated to every core.

Per-core design (v2, all-bf16 matmuls):
  The kernel is HBM-bound: 16.8 MB of x in + 16.8 MB of y out per core
  (~91 us at the ~370 GB/s a core sustains).  Everything else is scheduled
  to hide under that DMA stream:

  - x loads (fp32, [128, 1024] pieces) are the first instructions on the
    sync HWDGE ring; y stores ride the ACT HWDGE ring so they drain as
    computed and never head-of-line-block a load.
  - The PE runs every matmul in bf16 (1 cycle/row vs fp32r's ~1.5, and
    roughly half the power -> less HAM/SW clock throttling).  The idle
    Pool (gpsimd) engine casts each arriving fp32 x piece to a rotating
    bf16 tile right behind the DMA; x stays resident in fp32 for the
    exact residual add.
  - A burst of dummy PE transposes at t~0 keeps the PE busy until the
    first x piece lands, flipping the HAM clock gate (1.2 -> 2.4 GHz)
    before the real matmuls start.
  - Evacuations: ACT does q-bias evac, softmax exp, att@q evac and the
    store issues; DVE does qT copies, softmax vector ops and the
    fp32-exact residual adds (PSUM + x -> fin).
  - sample 0's step5 is interleaved with sample 1's stream phase; each
    unit emits [step5 matmuls+adds] -> [stream row] -> [deferred stores]
    so the ACT ring serves the stream's q evacuation before fin-gated
    stores.
"""

import os
import sys
from contextlib import ExitStack

import numpy as np

for _p in ("/opt/trn_rl_repo", "/root/.axon_site/_ro/trn_rl_repo"):
    if os.path.isdir(_p) and _p not in sys.path:
        sys.path.append(_p)

import concourse.bass as bass  # noqa: E402
import concourse.tile as tile  # noqa: E402
from concourse import bacc, mybir  # noqa: E402
from concourse.bass_utils import run_bass_kernel_spmd  # noqa: E402
from concourse.masks import make_identity  # noqa: E402

F32 = mybir.dt.float32
BF16 = mybir.dt.bfloat16
AF = mybir.ActivationFunctionType
ALU = mybir.AluOpType
AX = mybir.AxisListType

B, C, CR = 16, 512, 64
W, H = 64, 64
N = W * H  # 4096
NCORES = 8
BPC = B // NCORES  # samples per core
KC = C // 128  # 4 k-chunks of x / o-chunks of output
NF = 512  # moving-dim tile for big matmuls
NN = N // NF  # 8 n-chunks
NT = N // 128  # 32 transpose chunks
LF = 1024  # DMA piece width (load and store)
NL = N // LF  # 4 DMA pieces per chunk row
BPR = LF // NF  # n-blocks per piece row
WARMUP = 20  # dummy PE transposes to flip the HAM clock gate


def _build_nc():
    nc = bacc.Bacc(
        "TRN2",
        target_bir_lowering=False,
        debug=False,
        enable_asserts=True,
        num_devices=NCORES,
    )
    x_d = nc.dram_tensor("x", [BPC, C, N], F32, kind="ExternalInput").ap()
    w1_d = nc.dram_tensor("w1", [CR, C], F32, kind="ExternalInput").ap()
    b1_d = nc.dram_tensor("b1", [CR], F32, kind="ExternalInput").ap()
    w2_d = nc.dram_tensor("w2", [C, CR], F32, kind="ExternalInput").ap()
    b2_d = nc.dram_tensor("b2", [C], F32, kind="ExternalInput").ap()
    out_d = nc.dram_tensor("out", [BPC, C, N], F32, kind="ExternalOutput").ap()

    with tile.TileContext(nc) as tc, ExitStack() as ctx:
        singles = ctx.enter_context(tc.tile_pool(name="singles", bufs=1))
        xp = ctx.enter_context(tc.tile_pool(name="xp", bufs=2))
        xbf = ctx.enter_context(tc.tile_pool(name="xbf", bufs=2))
        qp = ctx.enter_context(tc.tile_pool(name="qp", bufs=2))
        qtp = ctx.enter_context(tc.tile_pool(name="qtp", bufs=2))
        oap = ctx.enter_context(tc.tile_pool(name="oap", bufs=2))
        fin = ctx.enter_context(tc.tile_pool(name="fin", bufs=4))
        small = ctx.enter_context(tc.tile_pool(name="small", bufs=2))
        ps_mm = ctx.enter_context(tc.tile_pool(name="ps_mm", bufs=2, space="PSUM"))
        ps_tp = ctx.enter_context(tc.tile_pool(name="ps_tp", bufs=2, space="PSUM"))
        ps_att = ctx.enter_context(tc.tile_pool(name="ps_att", bufs=1, space="PSUM"))
        ps_o = ctx.enter_context(tc.tile_pool(name="ps_o", bufs=3, space="PSUM"))

        # ---------- all x loads first on the sync ring ----------
        xts = []
        for s in range(BPC):
            xt = [
                xp.tile([128, N], F32, tag=f"x{k}", name=f"x{s}_{k}")
                for k in range(KC)
            ]
            for j in range(NL):
                lsl = bass.ts(j, LF)
                for k in range(KC):
                    nc.sync.dma_start(
                        out=xt[k][:, lsl],
                        in_=x_d[s, k * 128 : (k + 1) * 128, lsl],
                    )
            xts.append(xt)

        # ---------- weight loads on the ACT ring ----------
        w1_sb = singles.tile([CR, C], F32, tag="w1")  # [64, 512] natural
        nc.scalar.dma_start(out=w1_sb, in_=w1_d)
        b1_sb = singles.tile([CR, 1], F32, tag="b1")
        nc.scalar.dma_start(out=b1_sb, in_=b1_d.rearrange("(c one) -> c one", one=1))
        w2cs = []
        for oc in range(KC):
            w2c = small.tile([128, CR], F32, tag="w2chunk", name=f"w2c{oc}")
            nc.scalar.dma_start(out=w2c, in_=w2_d[oc * 128 : (oc + 1) * 128, :])
            w2cs.append(w2c)

        # ---------- identities (Pool) ----------
        ident = singles.tile([128, 128], F32, tag="ident")
        make_identity(nc, ident)
        identB = singles.tile([128, 128], BF16, tag="identB")
        nc.gpsimd.tensor_copy(identB, ident)

        # ---------- PE warm-up: flip HAM before the first x piece lands ----
        for wi in range(WARMUP):
            pw = ps_tp.tile([128, 128], F32, tag="tp", name=f"warm{wi}")
            nc.tensor.transpose(pw, ident, ident)

        # ---------- weight prep ----------
        # w1T: [512, 64] as [128, 4, 64] bf16 (chunk k = w1[:, 128k:+128].T)
        w1T = singles.tile([128, KC, CR], BF16, tag="w1T")
        for k in range(KC):
            ptp = ps_tp.tile([128, CR], F32, tag="tp", name=f"w1tp{k}")
            nc.tensor.transpose(ptp, w1_sb[:, k * 128 : (k + 1) * 128], ident[0:CR, 0:CR])
            nc.vector.tensor_copy(w1T[:, k, :], ptp)

        # w2aug: [65, 512] bf16; rows 0..63 = w2.T, row 64 = b2
        w2aug = singles.tile([CR + 1, C], BF16, tag="w2aug")
        for oc in range(KC):
            ptp = ps_tp.tile([CR, 128], F32, tag="tp", name=f"w2tp{oc}")
            nc.tensor.transpose(ptp, w2cs[oc], ident)
            nc.vector.tensor_copy(w2aug[0:CR, oc * 128 : (oc + 1) * 128], ptp)
        # b2 row via casting SWDGE DMA (gpsimd is the only engine that casts)
        nc.gpsimd.dma_start(
            out=w2aug[CR : CR + 1, :],
            in_=b2_d.rearrange("(one c) -> one c", one=1),
        )

        # ---------- per-sample phases ----------
        state = {}

        def begin_sample(s):
            oa = oap.tile([CR + 1, N], BF16, tag="oa", name=f"oa{s}")
            nc.gpsimd.memset(oa[CR : CR + 1, :], 1.0)
            state[s] = {
                "q": qp.tile([CR, N], BF16, tag="q", name=f"q{s}"),
                "qT": qtp.tile([128, NT, CR], BF16, tag="qT", name=f"qT{s}"),
                "patt": ps_att.tile([CR, CR], F32, tag="att", name=f"att{s}"),
                "oa": oa,
            }

        def stream_rows(s, j_lo, j_hi):
            """casts + q matmuls + transposes + att-Gram for rows [j_lo, j_hi)."""
            st = state[s]
            q, qT, patt = st["q"], st["qT"], st["patt"]
            xt = xts[s]
            for j in range(j_lo, j_hi):
                jsl = bass.ts(j, LF)
                xbs = []
                for k in range(KC):
                    xb = xbf.tile([128, LF], BF16, tag=f"xb{k}", name=f"xb{s}_{k}_{j}")
                    nc.gpsimd.tensor_copy(xb, xt[k][:, jsl])
                    xbs.append(xb)
                for h in range(BPR):
                    n = j * BPR + h
                    hsl = bass.ts(h, NF)
                    nsl = bass.ts(n, NF)
                    pq = ps_mm.tile([CR, NF], F32, tag="mm", name=f"pq{s}_{n}")
                    for k in range(KC):
                        nc.tensor.matmul(
                            pq, w1T[:, k, :], xbs[k][:, hsl],
                            start=(k == 0), stop=(k == KC - 1),
                        )
                    nc.scalar.activation(
                        q[:, nsl], pq, AF.Identity, bias=b1_sb, scale=1.0
                    )
                    for t_i in range(4 * n, 4 * n + 4):
                        ptp = ps_tp.tile([128, CR], BF16, tag="tp", name=f"tp{s}_{t_i}")
                        nc.tensor.transpose(
                            ptp,
                            q[:, t_i * 128 : (t_i + 1) * 128],
                            identB[0:CR, 0:CR],
                        )
                        nc.vector.tensor_copy(qT[:, t_i, :], ptp)
                        qTs = qT[:, t_i, :]
                        nc.tensor.matmul(
                            patt, qTs, qTs, start=(t_i == 0), stop=(t_i == NT - 1)
                        )

        def softmax_step4(s):
            st = state[s]
            q, patt, oa = st["q"], st["patt"], st["oa"]
            negm = small.tile([CR, 1], F32, tag="negm", name=f"negm{s}")
            nc.vector.tensor_reduce(
                out=negm, in_=patt, axis=AX.X, op=ALU.max, negate=True
            )
            shifted = small.tile([CR, CR], F32, tag="shifted", name=f"shifted{s}")
            nc.vector.tensor_scalar(
                out=shifted, in0=patt, scalar1=negm, scalar2=-80.0,
                op0=ALU.add, op1=ALU.max,
            )
            atte = small.tile([CR, CR], F32, tag="atte", name=f"atte{s}")
            ssum = small.tile([CR, 1], F32, tag="ssum", name=f"ssum{s}")
            nc.scalar.activation(
                atte, shifted, AF.Exp, bias=0.0, scale=1.0, accum_out=ssum
            )
            rsum = small.tile([CR, 1], F32, tag="rsum", name=f"rsum{s}")
            nc.vector.reciprocal(rsum, ssum)
            attn = small.tile([CR, CR], BF16, tag="attn", name=f"attn{s}")
            nc.vector.tensor_scalar_mul(attn, atte, rsum)
            pattT = ps_tp.tile([CR, CR], BF16, tag="tp", name=f"pattT{s}")
            nc.tensor.transpose(pattT, attn, identB[0:CR, 0:CR])
            attT = small.tile([CR, CR], BF16, tag="attT", name=f"attT{s}")
            nc.vector.tensor_copy(attT, pattT)
            # step 4: out = att @ q -> rows 0..63 of oa (row 64 is const 1.0)
            for n in range(NN):
                nsl = bass.ts(n, NF)
                po = ps_mm.tile([CR, NF], F32, tag="mm", name=f"po{s}_{n}")
                nc.tensor.matmul(po, attT, q[:, nsl], start=True, stop=True)
                nc.scalar.copy(oa[0:CR, nsl], po)

        def step5_chunk(s, oc):
            """y[oc] = w2aug[oc] @ out_aug + x[oc] into fin tiles (no stores)."""
            st = state[s]
            oa = st["oa"]
            xt = xts[s]
            osl = slice(oc * 128, (oc + 1) * 128)
            fins = []
            for half in range(NL):
                f = fin.tile([128, LF], F32, tag="fin", name=f"fin{s}_{oc}_{half}")
                for sub in range(BPR):
                    n = half * BPR + sub
                    nsl = bass.ts(n, NF)
                    p5 = ps_o.tile([128, NF], F32, tag="o5", name=f"p5{s}_{oc}_{n}")
                    nc.tensor.matmul(
                        p5, w2aug[:, osl], oa[:, nsl], start=True, stop=True
                    )
                    nc.vector.tensor_add(
                        f[:, bass.ts(sub, NF)], p5, xt[oc][:, nsl]
                    )
                fins.append((f, half))
            return fins

        def issue_stores(s, oc, fins):
            osl = slice(oc * 128, (oc + 1) * 128)
            for f, half in fins:
                nc.scalar.dma_start(out=out_d[s, osl, bass.ts(half, LF)], in_=f)

        # sample 0 stream + softmax
        begin_sample(0)
        stream_rows(0, 0, NL)
        softmax_step4(0)
        # interleave: s0 step5 chunks with s1 stream rows; stores deferred so
        # the ACT ring serves s1's q evacuation before the fin-gated stores
        begin_sample(1)
        for i in range(KC):
            fins = step5_chunk(0, i)
            stream_rows(1, i, i + 1)
            issue_stores(0, i, fins)
        softmax_step4(1)
        for i in range(KC):
            fins = step5_chunk(1, i)
            issue_stores(1, i, fins)

    nc.compile()
    return nc


_NC_CACHE = None


def _get_nc():
    global _NC_CACHE
    if _NC_CACHE is None:
        _NC_CACHE = _build_nc()
    return _NC_CACHE


def _as_f32(a):
    return np.ascontiguousarray(np.asarray(a, dtype=np.float32))


def run(inputs, trace=False):
    """Run on all 8 cores; returns (full output [B,C,W,H], BassKernelResults)."""
    nc = _get_nc()
    x = _as_f32(inputs["x"]).reshape(B, C, N)
    w1 = _as_f32(inputs["w1"])
    b1 = _as_f32(inputs["b1"])
    w2 = _as_f32(inputs["w2"])
    b2 = _as_f32(inputs["b2"])
    in_maps = [
        {
            "x": x[c * BPC : (c + 1) * BPC],
            "w1": w1,
            "b1": b1,
            "w2": w2,
            "b2": b2,
        }
        for c in range(NCORES)
    ]
    res = run_bass_kernel_spmd(nc, in_maps, list(range(NCORES)), trace=trace)
    out = np.concatenate([res.results[c]["out"] for c in range(NCORES)], axis=0)
    return out.reshape(B, C, W, H).astype(np.float32, copy=False), res


def kernel(**inputs):
    out, _ = run(inputs)
    return out


# revision 9
# speedup vs baseline: 1.2663x; 1.2663x over previous
"""Trainium2 Bass kernel for nn_AttentionModule (channel self-attention).

Reference computation (per batch sample b, with x: [C=512, N=4096]):
    q   = w1 @ x + b1                     # [64, 4096]
    att = softmax(q @ q.T, axis=-1)       # [64, 64]
    out = att @ q                         # [64, 4096]
    y   = w2 @ out + b2 + x               # [512, 4096]

Sharding: data-parallel over batch. B=16 samples, 8 cores, 2 samples/core.
Small weights (w1,b1,w2,b2) replicated to every core.

Per-core design (v2, all-bf16 matmuls):
  The kernel is HBM-bound: 16.8 MB of x in + 16.8 MB of y out per core
  (~91 us at the ~370 GB/s a core sustains).  Everything else is scheduled
  to hide under that DMA stream:

  - x loads (fp32, [128, 1024] pieces) are the first instructions on the
    sync HWDGE ring; y stores ride the ACT HWDGE ring so they drain as
    computed and never head-of-line-block a load.
  - The PE runs every matmul in bf16 (1 cycle/row vs fp32r's ~1.5, and
    roughly half the power -> less HAM/SW clock throttling).  The idle
    Pool (gpsimd) engine casts each arriving fp32 x piece to a rotating
    bf16 tile right behind the DMA; x stays resident in fp32 for the
    exact residual add.
  - A burst of dummy PE transposes at t~0 keeps the PE busy until the
    first x piece lands, flipping the HAM clock gate (1.2 -> 2.4 GHz)
    before the real matmuls start.
  - Evacuations: ACT does q-bias evac, softmax exp, att@q evac and the
    store issues; DVE does qT copies, softmax vector ops and the
    fp32-exact residual adds (PSUM + x -> fin).
  - sample 0's step5 is interleaved with sample 1's stream phase; each
    unit emits [step5 matmuls+adds] -> [stream row] -> [deferred stores]
    so the ACT ring serves the stream's q evacuation before fin-gated
    stores.
"""

import os
import sys
from contextlib import ExitStack

import numpy as np

for _p in ("/opt/trn_rl_repo", "/root/.axon_site/_ro/trn_rl_repo"):
    if os.path.isdir(_p) and _p not in sys.path:
        sys.path.append(_p)

import concourse.bass as bass  # noqa: E402
import concourse.tile as tile  # noqa: E402
from concourse import bacc, mybir  # noqa: E402
from concourse.bass_utils import run_bass_kernel_spmd  # noqa: E402
from concourse.masks import make_identity  # noqa: E402

F32 = mybir.dt.float32
F32R = mybir.dt.float32r
BF16 = mybir.dt.bfloat16
AF = mybir.ActivationFunctionType
ALU = mybir.AluOpType
AX = mybir.AxisListType

B, C, CR = 16, 512, 64
W, H = 64, 64
N = W * H  # 4096
NCORES = 8
BPC = B // NCORES  # samples per core
KC = C // 128  # 4 k-chunks of x / o-chunks of output
NF = 512  # moving-dim tile for big matmuls
NN = N // NF  # 8 n-chunks
NT = N // 128  # 32 transpose chunks
LF = 1024  # DMA piece width (load and store)
NL = N // LF  # 4 DMA pieces per chunk row
BPR = LF // NF  # n-blocks per piece row
WARMUP = 20  # dummy PE transposes to flip the HAM clock gate


def _build_nc():
    nc = bacc.Bacc(
        "TRN2",
        target_bir_lowering=False,
        debug=False,
        enable_asserts=True,
        num_devices=NCORES,
    )
    x_d = nc.dram_tensor("x", [BPC, C, N], F32, kind="ExternalInput").ap()
    w1_d = nc.dram_tensor("w1", [CR, C], F32, kind="ExternalInput").ap()
    b1_d = nc.dram_tensor("b1", [CR], F32, kind="ExternalInput").ap()
    w2_d = nc.dram_tensor("w2", [C, CR], F32, kind="ExternalInput").ap()
    b2_d = nc.dram_tensor("b2", [C], F32, kind="ExternalInput").ap()
    out_d = nc.dram_tensor("out", [BPC, C, N], F32, kind="ExternalOutput").ap()

    with tile.TileContext(nc) as tc, ExitStack() as ctx:
        singles = ctx.enter_context(tc.tile_pool(name="singles", bufs=1))
        xp = ctx.enter_context(tc.tile_pool(name="xp", bufs=2))
        qp = ctx.enter_context(tc.tile_pool(name="qp", bufs=2))
        qtp = ctx.enter_context(tc.tile_pool(name="qtp", bufs=2))
        oap = ctx.enter_context(tc.tile_pool(name="oap", bufs=2))
        fin = ctx.enter_context(tc.tile_pool(name="fin", bufs=4))
        small = ctx.enter_context(tc.tile_pool(name="small", bufs=2))
        ps_mm = ctx.enter_context(tc.tile_pool(name="ps_mm", bufs=2, space="PSUM"))
        ps_tp = ctx.enter_context(tc.tile_pool(name="ps_tp", bufs=2, space="PSUM"))
        ps_att = ctx.enter_context(tc.tile_pool(name="ps_att", bufs=1, space="PSUM"))
        ps_o = ctx.enter_context(tc.tile_pool(name="ps_o", bufs=3, space="PSUM"))

        # ---------- all x loads first on the sync ring ----------
        xts = []
        for s in range(BPC):
            xt = [
                xp.tile([128, N], F32R, tag=f"x{k}", name=f"x{s}_{k}")
                for k in range(KC)
            ]
            for j in range(NL):
                lsl = bass.ts(j, LF)
                for k in range(KC):
                    nc.sync.dma_start(
                        out=xt[k][:, lsl],
                        in_=x_d[s, k * 128 : (k + 1) * 128, lsl].bitcast(F32R),
                    )
            xts.append(xt)

        # ---------- weight loads on the ACT ring ----------
        w1_sb = singles.tile([CR, C], F32, tag="w1")  # [64, 512] natural
        nc.scalar.dma_start(out=w1_sb, in_=w1_d)
        b1_sb = singles.tile([CR, 1], F32, tag="b1")
        nc.scalar.dma_start(out=b1_sb, in_=b1_d.rearrange("(c one) -> c one", one=1))
        w2cs = []
        for oc in range(KC):
            w2c = small.tile([128, CR], F32, tag="w2chunk", name=f"w2c{oc}")
            nc.scalar.dma_start(out=w2c, in_=w2_d[oc * 128 : (oc + 1) * 128, :])
            w2cs.append(w2c)

        # ---------- identities (Pool) ----------
        ident = singles.tile([128, 128], F32, tag="ident")
        make_identity(nc, ident)
        identB = singles.tile([128, 128], BF16, tag="identB")
        nc.gpsimd.tensor_copy(identB, ident)

        # ---------- PE warm-up: flip HAM before the first x piece lands ----
        for wi in range(WARMUP):
            pw = ps_tp.tile([128, 128], F32, tag="tp", name=f"warm{wi}")
            nc.tensor.transpose(pw, ident, ident)

        # ---------- weight prep ----------
        # w1T: [512, 64] as [128, 4, 64] f32r (chunk k = w1[:, 128k:+128].T)
        w1T = singles.tile([128, KC, CR], F32R, tag="w1T")
        for k in range(KC):
            ptp = ps_tp.tile([128, CR], F32, tag="tp", name=f"w1tp{k}")
            nc.tensor.transpose(ptp, w1_sb[:, k * 128 : (k + 1) * 128], ident[0:CR, 0:CR])
            nc.vector.tensor_copy(w1T[:, k, :], ptp)

        # w2aug: [65, 512] bf16; rows 0..63 = w2.T, row 64 = b2
        w2aug = singles.tile([CR + 1, C], BF16, tag="w2aug")
        for oc in range(KC):
            ptp = ps_tp.tile([CR, 128], F32, tag="tp", name=f"w2tp{oc}")
            nc.tensor.transpose(ptp, w2cs[oc], ident)
            nc.vector.tensor_copy(w2aug[0:CR, oc * 128 : (oc + 1) * 128], ptp)
        # b2 row via casting SWDGE DMA (gpsimd is the only engine that casts)
        nc.gpsimd.dma_start(
            out=w2aug[CR : CR + 1, :],
            in_=b2_d.rearrange("(one c) -> one c", one=1),
        )

        # ---------- per-sample phases ----------
        state = {}

        def begin_sample(s):
            oa = oap.tile([CR + 1, N], BF16, tag="oa", name=f"oa{s}")
            nc.gpsimd.memset(oa[CR : CR + 1, :], 1.0)
            state[s] = {
                "q": qp.tile([CR, N], BF16, tag="q", name=f"q{s}"),
                "qT": qtp.tile([128, NT, CR], BF16, tag="qT", name=f"qT{s}"),
                "patt": ps_att.tile([CR, CR], F32, tag="att", name=f"att{s}"),
                "oa": oa,
            }

        def stream_rows(s, j_lo, j_hi):
            """q matmuls + transposes + att-Gram for rows [j_lo, j_hi)."""
            st = state[s]
            q, qT, patt = st["q"], st["qT"], st["patt"]
            xt = xts[s]
            for j in range(j_lo, j_hi):
                for h in range(BPR):
                    n = j * BPR + h
                    nsl = bass.ts(n, NF)
                    pq = ps_mm.tile([CR, NF], F32, tag="mm", name=f"pq{s}_{n}")
                    for k in range(KC):
                        nc.tensor.matmul(
                            pq, w1T[:, k, :], xt[k][:, nsl],
                            start=(k == 0), stop=(k == KC - 1),
                        )
                    nc.scalar.activation(
                        q[:, nsl], pq, AF.Identity, bias=b1_sb, scale=1.0
                    )
                    for t_i in range(4 * n, 4 * n + 4):
                        ptp = ps_tp.tile([128, CR], BF16, tag="tp", name=f"tp{s}_{t_i}")
                        nc.tensor.transpose(
                            ptp,
                            q[:, t_i * 128 : (t_i + 1) * 128],
                            identB[0:CR, 0:CR],
                        )
                        nc.vector.tensor_copy(qT[:, t_i, :], ptp)
                        qTs = qT[:, t_i, :]
                        nc.tensor.matmul(
                            patt, qTs, qTs, start=(t_i == 0), stop=(t_i == NT - 1)
                        )

        def softmax_step4(s):
            st = state[s]
            q, patt, oa = st["q"], st["patt"], st["oa"]
            negm = small.tile([CR, 1], F32, tag="negm", name=f"negm{s}")
            nc.vector.tensor_reduce(
                out=negm, in_=patt, axis=AX.X, op=ALU.max, negate=True
            )
            shifted = small.tile([CR, CR], F32, tag="shifted", name=f"shifted{s}")
            nc.vector.tensor_scalar(
                out=shifted, in0=patt, scalar1=negm, scalar2=-80.0,
                op0=ALU.add, op1=ALU.max,
            )
            atte = small.tile([CR, CR], F32, tag="atte", name=f"atte{s}")
            ssum = small.tile([CR, 1], F32, tag="ssum", name=f"ssum{s}")
            nc.scalar.activation(
                atte, shifted, AF.Exp, bias=0.0, scale=1.0, accum_out=ssum
            )
            rsum = small.tile([CR, 1], F32, tag="rsum", name=f"rsum{s}")
            nc.vector.reciprocal(rsum, ssum)
            attn = small.tile([CR, CR], BF16, tag="attn", name=f"attn{s}")
            nc.vector.tensor_scalar_mul(attn, atte, rsum)
            pattT = ps_tp.tile([CR, CR], BF16, tag="tp", name=f"pattT{s}")
            nc.tensor.transpose(pattT, attn, identB[0:CR, 0:CR])
            attT = small.tile([CR, CR], BF16, tag="attT", name=f"attT{s}")
            nc.vector.tensor_copy(attT, pattT)
            # step 4: out = att @ q -> rows 0..63 of oa (row 64 is const 1.0)
            for n in range(NN):
                nsl = bass.ts(n, NF)
                po = ps_mm.tile([CR, NF], F32, tag="mm", name=f"po{s}_{n}")
                nc.tensor.matmul(po, attT, q[:, nsl], start=True, stop=True)
                nc.scalar.copy(oa[0:CR, nsl], po)

        def step5_chunk(s, oc):
            """y[oc] = w2aug[oc] @ out_aug + x[oc] into fin tiles (no stores)."""
            st = state[s]
            oa = st["oa"]
            xt = xts[s]
            osl = slice(oc * 128, (oc + 1) * 128)
            fins = []
            for half in range(NL):
                f = fin.tile([128, LF], F32, tag="fin", name=f"fin{s}_{oc}_{half}")
                for sub in range(BPR):
                    n = half * BPR + sub
                    nsl = bass.ts(n, NF)
                    p5 = ps_o.tile([128, NF], F32, tag="o5", name=f"p5{s}_{oc}_{n}")
                    nc.tensor.matmul(
                        p5, w2aug[:, osl], oa[:, nsl], start=True, stop=True
                    )
                    nc.vector.tensor_add(
                        f[:, bass.ts(sub, NF)], p5, xt[oc][:, nsl].bitcast(F32)
                    )
                fins.append((f, half))
            return fins

        def issue_stores(s, oc, fins):
            osl = slice(oc * 128, (oc + 1) * 128)
            for f, half in fins:
                nc.scalar.dma_start(out=out_d[s, osl, bass.ts(half, LF)], in_=f)

        # sample 0 stream + softmax
        begin_sample(0)
        stream_rows(0, 0, NL)
        softmax_step4(0)
        # interleave: s0 step5 chunks with s1 stream rows; stores deferred so
        # the ACT ring serves s1's q evacuation before the fin-gated stores
        begin_sample(1)
        for i in range(KC):
            fins = step5_chunk(0, i)
            stream_rows(1, i, i + 1)
            issue_stores(0, i, fins)
        softmax_step4(1)
        for i in range(KC):
            fins = step5_chunk(1, i)
            issue_stores(1, i, fins)

    nc.compile()
    return nc


_NC_CACHE = None


def _get_nc():
    global _NC_CACHE
    if _NC_CACHE is None:
        _NC_CACHE = _build_nc()
    return _NC_CACHE


def _as_f32(a):
    return np.ascontiguousarray(np.asarray(a, dtype=np.float32))


def run(inputs, trace=False):
    """Run on all 8 cores; returns (full output [B,C,W,H], BassKernelResults)."""
    nc = _get_nc()
    x = _as_f32(inputs["x"]).reshape(B, C, N)
    w1 = _as_f32(inputs["w1"])
    b1 = _as_f32(inputs["b1"])
    w2 = _as_f32(inputs["w2"])
    b2 = _as_f32(inputs["b2"])
    in_maps = [
        {
            "x": x[c * BPC : (c + 1) * BPC],
            "w1": w1,
            "b1": b1,
            "w2": w2,
            "b2": b2,
        }
        for c in range(NCORES)
    ]
    res = run_bass_kernel_spmd(nc, in_maps, list(range(NCORES)), trace=trace)
    out = np.concatenate([res.results[c]["out"] for c in range(NCORES)], axis=0)
    return out.reshape(B, C, W, H).astype(np.float32, copy=False), res


def kernel(**inputs):
    out, _ = run(inputs)
    return out
